# revision 8
# baseline (speedup 1.0000x reference)
"""nn_MemoryAttention TP8 Trainium2 kernel.

8 NeuronCores, Megatron tensor-parallel, T-layout activations [feature, token]
(512 token cols = 4 batch x 128). Per-core shards: wq/wk/wv/wkm/wvm col-split
by heads (2/core), wm/wo col-split (256 feat), w1/w3 col-split (704->768 pad),
w2 row-split. Per chunk: wm -> AG -> rmsnorm -> SwiGLU FFN -> AR (om1/8
residual folded) -> rmsnorm -> mem K/V + RoPE -> attention (chunk queries
only) -> AG of attention out = next om. RoPE via head-dim permutation baked
into weight columns. All matmuls float32r.
"""
import numpy as np

DIM = 2048; NH = 16; HD = 128; MEM = 128; SEQ = 2048; B = 4; HID = 5632
EPS = 1e-5
NC = 8
HPC = NH // NC          # 2 heads per core
FPC = DIM // NC         # 256
HIDP = 768              # padded per-core hidden
NCH = SEQ // MEM        # 16 chunks
TOK = B * MEM           # 512 token columns
KT = DIM // 128         # 16 feature k-tiles
HKT = HIDP // 128       # 6 hidden k-tiles

_RUNTIME = {}


def _trunc22(x):
    return (np.ascontiguousarray(x, np.float32).view(np.uint32)
            & np.uint32(0xFFFFFC00)).view(np.float32)


def _head_perm():
    p = np.concatenate([np.arange(0, HD, 2), np.arange(1, HD, 2)])
    return np.concatenate([h * HD + p for h in range(NH)])


def _slab(w):
    """[2048, C] -> [C//128, 128, KT, 128]."""
    C = w.shape[1]
    return np.ascontiguousarray(
        w.reshape(KT, 128, C // 128, 128).transpose(2, 1, 0, 3))


def _slab_w2(w2):
    """[768, 2048] -> [16, 128, HKT, 128]."""
    return np.ascontiguousarray(
        w2.reshape(HKT, 128, 16, 128).transpose(2, 1, 0, 3))


def _prepare(inputs):
    perm = _head_perm()
    scale = 1.0 / np.sqrt(HD)
    wq = np.asarray(inputs["wq"])[:, perm] * scale
    wk = np.asarray(inputs["wk"])[:, perm]
    wkm = np.asarray(inputs["wkm"])[:, perm]
    wv = np.asarray(inputs["wv"]); wvm = np.asarray(inputs["wvm"])
    wm = np.asarray(inputs["wm"]); wo = np.asarray(inputs["wo"])
    w1 = np.zeros((DIM, NC * HIDP), np.float32)
    w3 = np.zeros((DIM, NC * HIDP), np.float32)
    w2 = np.zeros((NC * HIDP, DIM), np.float32)
    for c in range(NC):
        w1[:, c * HIDP:c * HIDP + 704] = np.asarray(inputs["w1"])[:, c * 704:(c + 1) * 704]
        w3[:, c * HIDP:c * HIDP + 704] = np.asarray(inputs["w3"])[:, c * 704:(c + 1) * 704]
        w2[c * HIDP:c * HIDP + 704] = np.asarray(inputs["w2"])[c * 704:(c + 1) * 704]
    fc = np.asarray(inputs["freqs_cos"]); fs = np.asarray(inputs["freqs_sin"])
    cos_mem = _trunc22(np.tile(fc[0:MEM].T, (1, B)))      # [64, 512]
    sin_mem = _trunc22(np.tile(fs[0:MEM].T, (1, B)))
    cos_x = _trunc22(np.tile(fc[MEM:2 * MEM].T, (1, B)))
    sin_x = _trunc22(np.tile(fs[MEM:2 * MEM].T, (1, B)))
    mask = np.zeros((MEM, 2 * MEM), np.float32)
    for i in range(MEM):
        mask[i, MEM + i + 1:] = -1e30
    x = np.asarray(inputs["x"])
    xT = _trunc22(x.reshape(B, NCH, MEM, DIM).transpose(1, 3, 0, 2)
                  .reshape(NCH, DIM, TOK))
    om0 = np.asarray(inputs["origin_mem"])
    omT0 = _trunc22(om0.transpose(2, 0, 1).reshape(DIM, TOK))
    fw = _trunc22(np.asarray(inputs["ffn_norm_w"]).reshape(KT, 128).T)  # [128,KT]
    mw = _trunc22(np.asarray(inputs["mem_norm_w"]).reshape(KT, 128).T)
    in_maps = []
    for c in range(NC):
        hsl = slice(c * FPC, (c + 1) * FPC)
        hidsl = slice(c * HIDP, (c + 1) * HIDP)
        in_maps.append({
            "WQ": _trunc22(_slab(wq[:, hsl])),
            "WK": _trunc22(_slab(wk[:, hsl])),
            "WKM": _trunc22(_slab(wkm[:, hsl])),
            "WM": _trunc22(_slab(wm[:, hsl])),
            "WO": _trunc22(_slab(wo[:, hsl])),
            "W1": _trunc22(_slab(w1[:, hidsl])),
            "W3": _trunc22(_slab(w3[:, hidsl])),
            "W2": _trunc22(_slab_w2(w2[hidsl, :])),
            "WV": _trunc22(np.ascontiguousarray(wv[:, hsl])),
            "WVM": _trunc22(np.ascontiguousarray(wvm[:, hsl])),
            "XT": xT, "OM0T": omT0,
            "COSM": cos_mem, "SINM": sin_mem, "COSX": cos_x, "SINX": sin_x,
            "MASK": mask, "FW": fw, "MW": mw,
        })
    return in_maps


def _build():
    import concourse.bacc as bacc
    import concourse.tile as tile
    import concourse.mybir as mybir
    from concourse.masks import make_identity
    from contextlib import ExitStack

    dt = mybir.dt
    AluOp = mybir.AluOpType
    AFT = mybir.ActivationFunctionType
    f32, f32r = dt.float32, dt.float32r

    nc = bacc.Bacc("TRN2", target_bir_lowering=False, debug=False,
                   num_devices=NC)

    def din(name, shape, dtype=f32r):
        return nc.dram_tensor(name, shape, dtype, kind="ExternalInput")

    WQ = din("WQ", [2, 128, KT, 128]); WK = din("WK", [2, 128, KT, 128])
    WKM = din("WKM", [2, 128, KT, 128]); WM = din("WM", [2, 128, KT, 128])
    WO = din("WO", [2, 128, KT, 128])
    W1 = din("W1", [HKT, 128, KT, 128]); W3 = din("W3", [HKT, 128, KT, 128])
    W2 = din("W2", [KT, 128, HKT, 128])
    WV = din("WV", [DIM, FPC]); WVM = din("WVM", [DIM, FPC])
    XT = din("XT", [NCH, DIM, TOK]); OM0T = din("OM0T", [DIM, TOK])
    COSM = din("COSM", [64, TOK], f32); SINM = din("SINM", [64, TOK], f32)
    COSX = din("COSX", [64, TOK], f32); SINX = din("SINX", [64, TOK], f32)
    MASK = din("MASK", [MEM, 2 * MEM], f32)
    FW = din("FW", [128, KT]); MW = din("MW", [128, KT])
    YO = nc.dram_tensor("YO", [NCH, FPC, TOK], f32, kind="ExternalOutput")

    rg = [list(range(NC))]

    with tile.TileContext(nc) as tc:
        es = ExitStack()
        const = es.enter_context(tc.tile_pool(name="const", bufs=1))
        wslab = es.enter_context(tc.tile_pool(name="wslab", bufs=2))
        w2slab = es.enter_context(tc.tile_pool(name="w2slab", bufs=2))
        wrhs = es.enter_context(tc.tile_pool(name="wrhs", bufs=2))
        xpool = es.enter_context(tc.tile_pool(name="xpool", bufs=1))
        big = es.enter_context(tc.tile_pool(name="big", bufs=2))
        gpool = es.enter_context(tc.tile_pool(name="gpool", bufs=1))
        qkpool = es.enter_context(tc.tile_pool(name="qkpool", bufs=2))
        vpool = es.enter_context(tc.tile_pool(name="vpool", bufs=2))
        kmpool = es.enter_context(tc.tile_pool(name="kmpool", bufs=1))
        stg = es.enter_context(tc.tile_pool(name="stg", bufs=3))
        attp = es.enter_context(tc.tile_pool(name="attp", bufs=1))
        tmpp = es.enter_context(tc.tile_pool(name="tmpp", bufs=1))
        smol = es.enter_context(tc.tile_pool(name="smol", bufs=1))
        dram = es.enter_context(tc.tile_pool(name="dram", bufs=1, space="DRAM"))
        psA = es.enter_context(tc.tile_pool(name="psA", bufs=3, space="PSUM"))
        psS = es.enter_context(tc.tile_pool(name="psS", bufs=2, space="PSUM"))
        psQ = es.enter_context(tc.tile_pool(name="psQ", bufs=1, space="PSUM"))

        # ---- constants
        cosm = const.tile([64, TOK], f32); nc.sync.dma_start(cosm[:], COSM[:])
        sinm = const.tile([64, TOK], f32); nc.sync.dma_start(sinm[:], SINM[:])
        cosx = const.tile([64, TOK], f32); nc.sync.dma_start(cosx[:], COSX[:])
        sinx = const.tile([64, TOK], f32); nc.sync.dma_start(sinx[:], SINX[:])
        maskt = const.tile([MEM, 2 * MEM], f32)
        nc.sync.dma_start(maskt[:], MASK[:])
        fwt = const.tile([128, KT], f32r); nc.sync.dma_start(fwt[:], FW[:])
        mwt = const.tile([128, KT], f32r); nc.sync.dma_start(mwt[:], MW[:])
        scratch32 = const.tile([128, 128], f32)
        nc.vector.memset(scratch32[:], 1.0)
        ones = const.tile([128, 1], f32r)
        nc.vector.tensor_copy(ones[:], scratch32[:, 0:1])
        onesrow = const.tile([1, 128], f32r)
        nc.vector.tensor_copy(onesrow[:], scratch32[0:1, :])
        ident32 = const.tile([128, 128], f32)
        make_identity(nc, ident32)
        ident = const.tile([128, 128], f32r)
        nc.vector.tensor_copy(ident[:], ident32[:])
        epst = const.tile([1, 1], f32)
        nc.vector.memset(epst[:], EPS)

        om = big.tile([128, KT, TOK], f32r, tag="big", name="om_init")
        nc.sync.dma_start(om[:], OM0T[:].rearrange("(k p) t -> p k t", p=128))

        def mm(p, lhsT, rhs, start, stop):
            nc.tensor.matmul(p, lhsT, rhs, start=start, stop=stop)

        def proj_nslab(Wd, rhs_tile):
            outs = []
            for n in range(2):
                ws = wslab.tile([128, KT, 128], f32r, tag="wslab", name="ws")
                nc.sync.dma_start(ws[:], Wd[n])
                p = psA.tile([128, TOK], f32, tag="mm", name="pp")
                for k in range(KT):
                    mm(p[:], ws[:, k, :], rhs_tile[:, k, :], k == 0, k == KT - 1)
                outs.append(p)
            return outs

        def rope2(dst, src01, cosT, sinT):
            for h in range(2):
                ph = src01[h]
                r, i = ph[0:64, :], ph[64:128, :]
                t1 = tmpp.tile([64, TOK], f32, tag="t1", name="t1")
                t2 = tmpp.tile([64, TOK], f32, tag="t2", name="t2")
                nc.vector.tensor_mul(t1[:], r, cosT[:])
                nc.vector.tensor_mul(t2[:], i, sinT[:])
                nc.vector.tensor_sub(dst[0:64, h, :], t1[:], t2[:])
                t3 = tmpp.tile([64, TOK], f32, tag="t1", name="t3")
                t4 = tmpp.tile([64, TOK], f32, tag="t2", name="t4")
                nc.vector.tensor_mul(t3[:], r, sinT[:])
                nc.vector.tensor_mul(t4[:], i, cosT[:])
                nc.vector.tensor_add(dst[64:128, h, :], t3[:], t4[:])

        def rmsnorm(src, w_tile, out):
            ssq = psQ.tile([1, TOK], f32, tag="ssq", name="ssq")
            for k in range(KT):
                sq = stg.tile([128, TOK], f32r, tag="scr", name="sq")
                nc.scalar.activation(sq[:], src[:, k, :], AFT.Square)
                mm(ssq[:], ones[:], sq[:], k == 0, k == KT - 1)
            rstd = smol.tile([1, TOK], f32, tag="rstd", name="rstd")
            nc.scalar.activation(rstd[:], ssq[:], AFT.Sqrt,
                                 bias=epst[:], scale=1.0 / DIM)
            rec = smol.tile([1, TOK], f32, tag="rec", name="rec")
            nc.vector.reciprocal(rec[:], rstd[:])
            rec_r = smol.tile([1, TOK], f32r, tag="recr", name="recr")
            nc.vector.tensor_copy(rec_r[:], rec[:])
            bc = psA.tile([128, TOK], f32, tag="mm", name="bc")
            mm(bc[:], onesrow[:], rec_r[:], True, True)
            for k in range(KT):
                nc.vector.scalar_tensor_tensor(
                    out[:, k, :], src[:, k, :], w_tile[:, k:k + 1], bc[:],
                    op0=AluOp.mult, op1=AluOp.mult)

        def vproj(lhs_tile, Wd, tag):
            """v (normal layout) [128 tok, B, FPC]; lhs = xk or om3."""
            v = vpool.tile([128, B, FPC], f32r, tag=tag, name=tag,
                           bufs=(2 if tag == "vx" else 1))
            pvs = [psS.tile([128, FPC], f32, tag=("sc" if b < 2 else "tr"),
                            name=f"pv{b}") for b in range(B)]
            for k in range(KT):
                wv_k = wrhs.tile([128, FPC], f32r, tag="wrhs", name="wvk")
                nc.sync.dma_start(wv_k[:], Wd[k * 128:(k + 1) * 128, :])
                for b in range(B):
                    mm(pvs[b][:], lhs_tile[:, k, b * 128:(b + 1) * 128],
                       wv_k[:], k == 0, k == KT - 1)
            for b in range(B):
                nc.scalar.copy(v[:, b, :], pvs[b][:])
            return v

        def x_side(t):
            xk = xpool.tile([128, KT, TOK], f32r, tag="xk", name="xk")
            nc.sync.dma_start(xk[:],
                              XT[t].rearrange("(k p) t2 -> p k t2", p=128))
            qps = proj_nslab(WQ, xk)
            qT = qkpool.tile([128, 2, TOK], f32r, tag="qT", name="qT")
            rope2(qT, qps, cosx, sinx)
            kps = proj_nslab(WK, xk)
            kxT = qkpool.tile([128, 2, TOK], f32r, tag="kxT", name="kxT")
            rope2(kxT, kps, cosx, sinx)
            vx = vproj(xk, WV, "vx")
            return qT, kxT, vx

        def yo_proj(om_tile, t_out):
            for n in range(2):
                ws = wslab.tile([128, KT, 128], f32r, tag="wslab", name="wos")
                nc.sync.dma_start(ws[:], WO[n])
                p = psA.tile([128, TOK], f32, tag="mm", name="pyo")
                for k in range(KT):
                    mm(p[:], ws[:, k, :], om_tile[:, k, :], k == 0, k == KT - 1)
                o = stg.tile([128, TOK], f32, tag="io", name="yo")
                nc.scalar.copy(o[:], p[:])
                nc.sync.dma_start(YO[t_out, n * 128:(n + 1) * 128, :], o[:])

        qT, kxT, vx = x_side(0)

        for t in range(NCH):
            if t > 0:
                yo_proj(om, t - 1)
            # ---- wm projection + AG1
            omps = proj_nslab(WM, om)
            agin1 = dram.tile([FPC, TOK], f32r, name=f"agin1_{t}")
            for n in range(2):
                s = stg.tile([128, TOK], f32r, tag="io", name="oml")
                nc.scalar.copy(s[:], omps[n][:])
                nc.sync.dma_start(agin1[n * 128:(n + 1) * 128, :], s[:])
            agout1 = dram.tile([DIM, TOK], f32r, addr_space="Shared",
                               name=f"agout1_{t}")
            nc.gpsimd.collective_compute(
                "AllGather", AluOp.bypass, replica_groups=rg,
                ins=[agin1[:].opt()], outs=[agout1[:].opt()])
            if t + 1 < NCH:
                qT_n, kxT_n, vx_n = x_side(t + 1)
            om1 = big.tile([128, KT, TOK], f32r, tag="big", name="om1")
            nc.sync.dma_start(om1[:],
                              agout1[:].rearrange("(k p) t2 -> p k t2", p=128))
            # ---- norm1 + FFN up
            h = big.tile([128, KT, TOK], f32r, tag="big", name="h")
            rmsnorm(om1, fwt, h)
            g = gpool.tile([128, HKT, TOK], f32r, tag="g", name="g")
            for n in range(HKT):
                w1s = wslab.tile([128, KT, 128], f32r, tag="wslab", name="w1s")
                nc.sync.dma_start(w1s[:], W1[n])
                w3s = wslab.tile([128, KT, 128], f32r, tag="wslab", name="w3s")
                nc.sync.dma_start(w3s[:], W3[n])
                p1 = psA.tile([128, TOK], f32, tag="mm", name="p1")
                p3 = psA.tile([128, TOK], f32, tag="mm", name="p3")
                for k in range(KT):
                    mm(p1[:], w1s[:, k, :], h[:, k, :], k == 0, k == KT - 1)
                for k in range(KT):
                    mm(p3[:], w3s[:, k, :], h[:, k, :], k == 0, k == KT - 1)
                sil = stg.tile([128, TOK], f32, tag="scr", name="sil")
                nc.scalar.activation(sil[:], p1[:], AFT.Silu)
                nc.vector.tensor_mul(g[:, n, :], sil[:], p3[:])
            # ---- FFN down + fused residual + AR
            arin = dram.tile([DIM, TOK], f32r, name=f"arin_{t}")
            for nf in range(KT):
                w2s = w2slab.tile([128, HKT, 128], f32r, tag="w2s", name="w2s")
                nc.sync.dma_start(w2s[:], W2[nf])
                p = psA.tile([128, TOK], f32, tag="mm", name="pd")
                for k in range(HKT):
                    mm(p[:], w2s[:, k, :], g[:, k, :], k == 0, k == HKT - 1)
                o = stg.tile([128, TOK], f32r, tag="scr", name="fo")
                nc.vector.scalar_tensor_tensor(
                    o[:], om1[:, nf, :], 1.0 / NC, p[:],
                    op0=AluOp.mult, op1=AluOp.add)
                nc.sync.dma_start(arin[nf * 128:(nf + 1) * 128, :], o[:])
            arout = dram.tile([DIM, TOK], f32r, addr_space="Shared",
                              name=f"arout_{t}")
            nc.gpsimd.collective_compute(
                "AllReduce", AluOp.add, replica_groups=rg,
                ins=[arin[:].opt()], outs=[arout[:].opt()])
            om2 = big.tile([128, KT, TOK], f32r, tag="big", name="om2")
            nc.sync.dma_start(om2[:],
                              arout[:].rearrange("(k p) t2 -> p k t2", p=128))
            # ---- norm2 + memory K/V
            om3 = big.tile([128, KT, TOK], f32r, tag="big", name="om3")
            rmsnorm(om2, mwt, om3)
            kmps = proj_nslab(WKM, om3)
            kmT = kmpool.tile([128, 2, TOK], f32r, tag="kmT", name="kmT")
            rope2(kmT, kmps, cosm, sinm)
            vm = vproj(om3, WVM, "vm")
            # ---- attention (chunk queries only)
            aout = stg.tile([128, 2, TOK], f32r, tag="aout", name="aout",
                            bufs=1)
            for h_ in range(HPC):
                for b in range(B):
                    bs = slice(b * 128, (b + 1) * 128)
                    ps = psS.tile([128, 2 * MEM], f32, tag="sc", name="ps")
                    mm(ps[:, 0:128], qT[:, h_, bs], kmT[:, h_, bs], True, True)
                    mm(ps[:, 128:256], qT[:, h_, bs], kxT[:, h_, bs], True, True)
                    s = attp.tile([128, 2 * MEM], f32, tag="s", name="s")
                    nc.vector.tensor_add(s[:], ps[:], maskt[:])
                    negmax = smol.tile([128, 1], f32, tag="negmax", name="nm")
                    nc.vector.tensor_reduce(negmax[:], s[:],
                                            mybir.AxisListType.X, AluOp.max,
                                            negate=True)
                    e = attp.tile([128, 2 * MEM], f32, tag="e", name="e")
                    den = smol.tile([128, 1], f32, tag="den", name="den")
                    nc.scalar.activation(e[:], s[:], AFT.Exp,
                                         bias=negmax[:], scale=1.0,
                                         accum_out=den[:])
                    rec = smol.tile([128, 1], f32, tag="rec2", name="rec2")
                    nc.vector.reciprocal(rec[:], den[:])
                    att = attp.tile([128, 2 * MEM], f32r, tag="att", name="att")
                    nc.vector.tensor_scalar_mul(att[:], e[:], rec[:])
                    attT = attp.tile([128, 2, 128], f32r, tag="attT",
                                     name="attT")
                    for half in range(2):
                        pt = psS.tile([128, 128], f32r, tag="tr", name="pt")
                        nc.tensor.transpose(
                            pt[:], att[:, half * 128:(half + 1) * 128],
                            ident[:])
                        nc.vector.tensor_copy(attT[:, half, :], pt[:])
                    po = psS.tile([128, 128], f32, tag="tr", name="po")
                    mm(po[:], vm[:, b, h_ * 128:(h_ + 1) * 128],
                       attT[:, 0, :], True, False)
                    mm(po[:], vx[:, b, h_ * 128:(h_ + 1) * 128],
                       attT[:, 1, :], False, True)
                    nc.scalar.copy(aout[:, h_, bs], po[:])
            # ---- AG3 -> next om
            agin3 = dram.tile([FPC, TOK], f32r, name=f"agin3_{t}")
            for h_ in range(HPC):
                nc.sync.dma_start(agin3[h_ * 128:(h_ + 1) * 128, :],
                                  aout[:, h_, :])
            agout3 = dram.tile([DIM, TOK], f32r, addr_space="Shared",
                               name=f"agout3_{t}")
            nc.gpsimd.collective_compute(
                "AllGather", AluOp.bypass, replica_groups=rg,
                ins=[agin3[:].opt()], outs=[agout3[:].opt()])
            om = big.tile([128, KT, TOK], f32r, tag="big", name=f"om_{t + 1}")
            nc.sync.dma_start(om[:],
                              agout3[:].rearrange("(k p) t2 -> p k t2", p=128))
            if t + 1 < NCH:
                qT, kxT, vx = qT_n, kxT_n, vx_n
        yo_proj(om, NCH - 1)
        es.close()

    nc.compile()
    return nc


def _get_runtime():
    if "nc" not in _RUNTIME:
        _RUNTIME["nc"] = _build()
    return _RUNTIME["nc"]


def _assemble(results):
    out = np.zeros((B, SEQ, DIM), np.float32)
    for c in range(NC):
        yo = results[c]["YO"]  # [NCH, FPC, TOK]
        y = yo.reshape(NCH, FPC, B, MEM).transpose(2, 0, 3, 1)
        out[:, :, c * FPC:(c + 1) * FPC] = y.reshape(B, SEQ, FPC)
    return out


def kernel(**inputs):
    from concourse.bass_utils import run_bass_kernel_spmd
    nc = _get_runtime()
    in_maps = _prepare(inputs)
    res = run_bass_kernel_spmd(nc, in_maps, core_ids=list(range(NC)),
                               trace=False)
    return _assemble(res.results)


if __name__ == "__main__":
    _build()
    print("build ok")
